# revision 1
# baseline (speedup 1.0000x reference)
import numpy as np, sys, os
sys.path.insert(0, "/opt/trn_rl_repo")
NO_COLL = os.environ.get("BASS_NO_COLL", "") == "1"
NLAYERS = int(os.environ.get("BASS_NLAYERS", "8"))
SKIP_ATTN = os.environ.get("BASS_SKIP_ATTN", "") == "1"
SKIP_FFN = os.environ.get("BASS_SKIP_FFN", "") == "1"
SKIP_LM = os.environ.get("BASS_SKIP_LM", "") == "1"
HALF_LGWR = os.environ.get("BASS_HALF_LGWR", "") == "1"

V, D, L = 32000, 1024, 8
HQ, HKV, HD = 16, 4, 64
H = 2752
HP = 2816                      # H padded to 22*128
B, S = 2, 1024
WINDOW, GEVERY = 256, 4
EPS, BASE = 1e-6, 10000.0
NCORES = 8
T = 256
VSH = V // NCORES
NEG = -30000.0
SCALE = 1.0 / 8.0

# kv gather order: col-chunk j holds absolute block PI[j]
PI = [0, 7, 1, 6, 2, 5, 3, 4]
JOF = [PI.index(b) for b in range(8)]   # absolute block -> gathered chunk
# head -> (qT tile, base) with base == (kvh%2)*64 so lhsT/rhs partitions align
APAR = [0, 1, 2, 3, 8, 9, 10, 11]       # kvh even -> base 0
BPAR = [4, 5, 6, 7, 12, 13, 14, 15]     # kvh odd  -> base 64
TILE = {}
for t in range(8):
    TILE[APAR[t]] = t
    TILE[BPAR[t]] = t
QPERM = []                               # head order within wq/wo layouts
for t in range(8):
    QPERM += [APAR[t], BPAR[t]]
ROPE_PERM = [2 * f for f in range(32)] + [2 * f + 1 for f in range(32)]

def _core_blocks(c):
    cp = c % 4
    return cp, 7 - cp

def _inv_freq():
    return 1.0 / (BASE ** (np.arange(0, HD, 2, dtype=np.float64) / HD))

def _host_masks(c):
    """Transposed block-masks [k_in_block, (block, q)] in gather-group layout."""
    bA, bB = _core_blocks(c)
    p = np.arange(128)
    def mk(qblk, blocks, local):
        cols = []
        for b in blocks:
            k = (b * 128 + p)[:, None]
            q = (qblk * 128 + p)[None, :]
            valid = k <= q
            if local:
                valid &= (q - k) < WINDOW
            cols.append(np.where(valid, 0.0, NEG))
        return np.concatenate(cols, axis=1).astype(np.float16)
    return (mk(bA, [0, 1, 2, 3], False),       # mAg [128,512]
            mk(bB, [4, 5, 6, 7], False),       # mBg [128,512] (blocks 0-3 maskless)
            mk(bA, [0, 1, 2, 3], True),        # mAl [128,512]
            mk(bB, [2, 3, 4, 5, 6, 7], True))  # mBl [128,768]

def _host_prep(idx, emb, Wq, Wk, Wv, Wo, w1, w3, w2, n1, n2, nf):
    idx = np.asarray(idx)
    emb = np.asarray(emb, dtype=np.float32)
    Wq = np.asarray(Wq, dtype=np.float32); Wk = np.asarray(Wk, dtype=np.float32)
    Wv = np.asarray(Wv, dtype=np.float32); Wo = np.asarray(Wo, dtype=np.float32)
    w1 = np.asarray(w1, dtype=np.float32); w3 = np.asarray(w3, dtype=np.float32)
    w2 = np.asarray(w2, dtype=np.float32)
    invf = _inv_freq()

    # wq: [D,1024] -> heads in QPERM order, rope-permuted cols -> [128, 8, 1024]
    wqv = Wq.reshape(L, D, HQ, HD)[:, :, QPERM][:, :, :, ROPE_PERM]
    wqh = np.ascontiguousarray(
        wqv.reshape(L, 8, 128, HQ * HD).transpose(0, 2, 1, 3).astype(np.float16))
    # wk: natural kvh order, rope-permuted cols
    wkv = Wk.reshape(L, D, HKV, HD)[:, :, :, ROPE_PERM]
    wkh = np.ascontiguousarray(
        wkv.reshape(L, 8, 128, HKV * HD).transpose(0, 2, 1, 3).astype(np.float16))
    wvh = np.ascontiguousarray(
        Wv.reshape(L, 8, 128, HKV * HD).transpose(0, 2, 1, 3).astype(np.float16))
    # wo rows permuted to QPERM head-block order (natural within head)
    wov = Wo.reshape(L, HQ, HD, D)[:, QPERM]
    woh = np.ascontiguousarray(
        wov.reshape(L, 8, 128, D).transpose(0, 2, 1, 3).astype(np.float16))
    # FFN padded to HP
    w1p = np.zeros((L, D, HP), np.float16); w1p[:, :, :H] = w1
    w3p = np.zeros((L, D, HP), np.float16); w3p[:, :, :H] = w3
    w2p = np.zeros((L, HP, D), np.float16); w2p[:, :H, :] = w2
    w1h = np.ascontiguousarray(w1p.reshape(L, 8, 128, HP).transpose(0, 2, 1, 3))
    w3h = np.ascontiguousarray(w3p.reshape(L, 8, 128, HP).transpose(0, 2, 1, 3))
    w2h = np.ascontiguousarray(w2p.reshape(L, 22, 128, D).transpose(0, 2, 1, 3))

    n1h = np.ascontiguousarray(np.asarray(n1).reshape(L, 8, 128).transpose(0, 2, 1)).astype(np.float32)
    n2h = np.ascontiguousarray(np.asarray(n2).reshape(L, 8, 128).transpose(0, 2, 1)).astype(np.float32)
    nfh = np.ascontiguousarray(np.asarray(nf).reshape(8, 128).T).astype(np.float32)

    in_maps = []
    for c in range(NCORES):
        s = c // 4
        bA, bB = _core_blocks(c)
        tok = np.concatenate([idx[s, bA*128:(bA+1)*128], idx[s, bB*128:(bB+1)*128]])
        x0T = np.ascontiguousarray(emb[tok].T)
        pos = np.concatenate([bA*128 + np.arange(128), bB*128 + np.arange(128)])
        ang = invf[:, None] * pos[None, :].astype(np.float64)       # [32, 256]
        cos32 = np.cos(ang).astype(np.float32)
        sin32 = np.sin(ang).astype(np.float32)
        cs = np.concatenate([cos32, cos32, cos32, cos32], axis=0)   # [128,256]
        sc = np.concatenate([-sin32, sin32, -sin32, sin32], axis=0)
        mAg, mBg, mAl, mBl = _host_masks(c)
        embT = np.ascontiguousarray(
            emb[c*VSH:(c+1)*VSH].T.astype(np.float16).reshape(8, 128, VSH).transpose(1, 0, 2))
        in_maps.append({
            "x0T": x0T,
            "wq": wqh,
            "wk": wkh, "wv": wvh, "wo": woh,
            "w1": w1h, "w3": w3h, "w2": w2h,
            "n1h": n1h, "n2h": n2h, "nfh": nfh,
            "cs": cs, "sc": sc,
            "mAg": mAg, "mBg": mBg, "mAl": mAl, "mBl": mBl,
            "embT": embT,
        })
    return in_maps

def _unperm_rows():
    perm = np.zeros(2048, dtype=np.int64)
    for r in range(2048):
        rr, rem = divmod(r, 256)
        slot, p = divmod(rem, 128)
        samp = rr // 4
        bA, bB = _core_blocks(rr)
        blk = bA if slot == 0 else bB
        perm[r] = samp * S + blk * 128 + p
    inv = np.zeros(2048, dtype=np.int64)
    inv[perm] = np.arange(2048)
    return inv

def _assemble(outs):
    full = np.empty((2048, V), dtype=np.float32)
    for c in range(NCORES):
        full[:, c*VSH:(c+1)*VSH] = outs[c]["logits"].astype(np.float32)
    inv = _unperm_rows()
    return full[inv].reshape(B, S, V)

def _build_nc():
    import concourse.bass as bass
    import concourse.bacc as bacc
    import concourse.mybir as mybir
    from concourse.tile import TileContext
    F32, F16, F32R = mybir.dt.float32, mybir.dt.float16, mybir.dt.float32r
    AF = mybir.ActivationFunctionType
    ALU = mybir.AluOpType

    nc = bacc.Bacc("TRN2", target_bir_lowering=False, debug=False, num_devices=NCORES)
    P = {}
    def inp(name, shape, dt=F16):
        P[name] = nc.declare_dram_parameter(name, list(shape), dt, isOutput=False)
    inp("x0T", (D, T), F32)
    inp("wq", (L, 128, 8, 1024)); inp("wk", (L, 128, 8, 256)); inp("wv", (L, 128, 8, 256))
    inp("wo", (L, 128, 8, 1024))
    inp("w1", (L, 128, 8, HP)); inp("w3", (L, 128, 8, HP)); inp("w2", (L, 128, 22, 1024))
    inp("n1h", (L, 128, 8), F32); inp("n2h", (L, 128, 8), F32); inp("nfh", (128, 8), F32)
    inp("cs", (128, 256), F32); inp("sc", (128, 256), F32)
    inp("mAg", (128, 512)); inp("mBg", (128, 512))
    inp("mAl", (128, 512)); inp("mBl", (128, 768))
    inp("embT", (128, 8, VSH))
    logits = nc.declare_dram_parameter("logits", [2048, VSH], F16, isOutput=True)

    kv_in  = [nc.dram_tensor(f"kv_in{l}",  [4, 128, 256], F16) for l in range(L)]
    kv_out = [nc.dram_tensor(f"kv_out{l}", [16, 128, 256], F16) for l in range(L)]
    xf_in  = nc.dram_tensor("xf_in", [128, 2048], F16)
    xf_out = nc.dram_tensor("xf_out", [NCORES * 128, 2048], F16, addr_space="Shared")
    RG_KV = [[0, 1, 2, 3], [4, 5, 6, 7]]
    RG_ALL = [list(range(NCORES))]

    with TileContext(nc) as tc:
      with tc.tile_pool(name="pers", bufs=1) as pers, \
           tc.tile_pool(name="wpool", bufs=2) as wp, \
           tc.tile_pool(name="act", bufs=2) as act, \
           tc.tile_pool(name="attn", bufs=2) as atp, \
           tc.tile_pool(name="small", bufs=4) as sm, \
           tc.tile_pool(name="ppffn", bufs=4, space="PSUM") as ppm, \
           tc.tile_pool(name="pps", bufs=2, space="PSUM") as pps, \
           tc.tile_pool(name="ppo", bufs=2, space="PSUM") as ppo:

        dma = nc.sync.dma_start
        dma_a = nc.scalar.dma_start
        dma_g = nc.gpsimd.dma_start
        xT = [pers.tile([128, T], F32, tag=f"xT{d}", name=f"xT{d}") for d in range(8)]
        for d in range(8):
            dma_g(out=xT[d], in_=P["x0T"][d*128:(d+1)*128, :])
        cs = pers.tile([128, 256], F32, tag="cs", name="cs"); dma_g(out=cs, in_=P["cs"][:, :])
        sc = pers.tile([128, 256], F32, tag="sc", name="sc"); dma_g(out=sc, in_=P["sc"][:, :])
        mAg = pers.tile([128, 512], F16, tag="mAg", name="mAg"); dma_g(out=mAg, in_=P["mAg"][:, :])
        mBg = pers.tile([128, 512], F16, tag="mBg", name="mBg"); dma_g(out=mBg, in_=P["mBg"][:, :])
        mAl = pers.tile([128, 512], F16, tag="mAl", name="mAl"); dma_g(out=mAl, in_=P["mAl"][:, :])
        mBl = pers.tile([128, 768], F16, tag="mBl", name="mBl"); dma_g(out=mBl, in_=P["mBl"][:, :])
        onesf = pers.tile([128, 1], F32, tag="onesf", name="onesf")
        nc.vector.memset(onesf, 1.0)
        ones = pers.tile([128, 1], F32R, tag="ones", name="ones")
        nc.vector.tensor_copy(out=ones, in_=onesf)
        ones1f = pers.tile([1, 128], F32, tag="ones1f", name="ones1f")
        nc.vector.memset(ones1f, 1.0)
        ones1r = pers.tile([1, 128], F32R, tag="ones1r", name="ones1r")
        nc.vector.tensor_copy(out=ones1r, in_=ones1f)
        epst = pers.tile([1, 1], F32, tag="epst", name="epst")
        nc.vector.memset(epst, EPS)

        def rmsnorm(nw_dram):
            nw = sm.tile([128, 8], F32, tag="nw", name="nw")
            dma_g(out=nw, in_=nw_dram)
            ss = ppm.tile([1, T], F32, tag="pm", name="ss")
            for d in range(8):
                x2 = act.tile([128, T], F32R, tag="x2", name="x2")
                nc.vector.tensor_mul(out=x2, in0=xT[d], in1=xT[d])
                nc.tensor.matmul(ss, lhsT=ones, rhs=x2, start=(d == 0), stop=(d == 7))
            rrow = sm.tile([1, T], F32, tag="rrow", name="rrow", bufs=2)
            nc.scalar.activation(out=rrow, in_=ss, func=AF.Sqrt, scale=1.0/D, bias=epst[0:1, 0:1])
            rrec = sm.tile([1, T], F32R, tag="rrec", name="rrec", bufs=2)
            with nc.allow_low_precision(reason="rsqrt broadcast row, f32r for PE outer-product"):
                nc.vector.reciprocal(out=rrec, in_=rrow)
            rb = ppm.tile([128, T], F32, tag="pm", name="rb")
            nc.tensor.matmul(rb, lhsT=ones1r, rhs=rrec, start=True, stop=True)
            out = []
            for d in range(8):
                h = act.tile([128, T], F16, tag=f"hT{d}", name=f"hT{d}", bufs=1)
                nc.vector.scalar_tensor_tensor(out=h, in0=xT[d], scalar=nw[:, d:d+1],
                                               in1=rb, op0=ALU.mult, op1=ALU.mult)
                out.append(h)
            return out

        def rope_tile(ps, outt):
            """qt = ps*COS + swap32(ps)*SS;  rows = [e0,o0,e1,o1] 32-blocks."""
            swp = sm.tile([128, 256], F32, tag="ropeswp", name="ropeswp", bufs=2)
            for j in range(4):
                sj = j ^ 1
                nc.scalar.activation(out=swp[j*32:(j+1)*32, :],
                                     in_=ps[sj*32:(sj+1)*32, :], func=AF.Copy)
            u = sm.tile([128, 256], F32, tag="ropeu", name="ropeu", bufs=2)
            t = sm.tile([128, 256], F32, tag="ropet", name="ropet", bufs=2)
            nc.vector.tensor_mul(out=u, in0=ps, in1=cs)
            nc.vector.tensor_mul(out=t, in0=swp, in1=sc)
            nc.vector.tensor_add(out=outt, in0=u, in1=t)

        for l in range(NLAYERS):
            is_global = ((l + 1) % GEVERY) == 0
            h1 = rmsnorm(P["n1h"][l])
            # ---- K^T (direct), V ----
            wkt = wp.tile([128, 8, 256], F16, tag="wkt", name="wkt", bufs=1)
            wvt = wp.tile([128, 8, 256], F16, tag="wvt", name="wvt", bufs=1)
            dma(out=wkt, in_=P["wk"][l])
            dma(out=wvt, in_=P["wv"][l])
            for i in range(2):
                psk = ppm.tile([128, 256], F32, tag="pm", name="psk")
                for d in range(8):
                    nc.tensor.matmul(psk, lhsT=wkt[:, d, i*128:(i+1)*128], rhs=h1[d],
                                     start=(d == 0), stop=(d == 7))
                kt = atp.tile([128, 256], F16, tag=f"ktc{i}", name=f"ktc{i}")
                rope_tile(psk, kt)
                dma_g(out=kv_in[l][i], in_=kt)
            for t2_ in range(2):
                psv = ppm.tile([128, 256], F32, tag="pm", name="psv")
                for d in range(8):
                    nc.tensor.matmul(psv, lhsT=h1[d][:, t2_*128:(t2_+1)*128], rhs=wvt[:, d, :],
                                     start=(d == 0), stop=(d == 7))
                vt = atp.tile([128, 256], F16, tag=f"vtok{t2_}", name=f"vtok{t2_}")
                nc.vector.tensor_copy(out=vt, in_=psv)
                dma_g(out=kv_in[l][2 + t2_], in_=vt)
            if NO_COLL:
                for r_ in range(4):
                    dma_g(out=kv_out[l][r_*4:(r_+1)*4], in_=kv_in[l][:])
            else:
                nc.gpsimd.collective_compute(
                    "AllGather", mybir.AluOpType.bypass, replica_groups=RG_KV,
                    ins=[kv_in[l].ap()], outs=[kv_out[l].ap()])
            # ---- Q^T (direct) ----
            wqt = wp.tile([128, 8, 1024], F16, tag="wqt", name="wqt", bufs=1)
            dma_a(out=wqt, in_=P["wq"][l])
            qTk = [act.tile([128, 1024], F16, tag=f"qTk{i}", name=f"qTk{i}", bufs=1)
                   for i in range(2)]
            for t in range(8):
                i, j = t // 4, t % 4
                psq = ppm.tile([128, 256], F32, tag="pm", name="psq")
                for d in range(8):
                    nc.tensor.matmul(psq, lhsT=wqt[:, d, t*128:(t+1)*128], rhs=h1[d],
                                     start=(d == 0), stop=(d == 7))
                rope_tile(psq, qTk[i][:, j::4])
            # ---- gather K/V from collective ----
            kT_full = [atp.tile([128, 1024], F16, tag=f"kTf{i}", name=f"kTf{i}", bufs=1) for i in range(2)]
            for i in range(2):
                dma_g(out=kT_full[i].rearrange("p (r t) -> p r t", r=4),
                    in_=kv_out[l][i::4].rearrange("r p t -> p r t"))
            v_full = atp.tile([128, 8, 4, 128], F16, tag="vfull", name="vfull")
            for sl in range(2):
                for r in range(4):
                    dma_g(out=v_full[:, 2*r+sl, :, 0:64],
                        in_=kv_out[l][r*4+2+sl].rearrange("p (f h) -> p f h", h=64))
            nc.gpsimd.memset(v_full[:, :, :, 64:128], 1.0)
            # ---- attention (4 q heads batched per kv head) ----
            oTk = [act.tile([128, 1024], F16, tag=f"oTk{i}", name=f"oTk{i}", bufs=1)
                   for i in range(2)]
            for i in range(2 if not SKIP_ATTN else 0):
                for sl in range(2):
                    kvh = 2 * i + sl
                    base = sl * 64
                    for qb in range(2):
                        if qb == 0:
                            groups = [([0, 1, 2, 3], mAg if is_global else mAl)]
                        elif is_global:
                            groups = [([0, 1, 2, 3], None), ([4, 5, 6, 7], mBg)]
                        else:
                            groups = [([2, 3, 4, 5], mBl[:, 0:512]), ([6, 7], mBl[:, 512:768])]
                        nblk = sum(len(g[0]) for g in groups)
                        pts = []
                        for (blocks, msk) in groups:
                            for gi, b in enumerate(blocks):
                                psS = pps.tile([128, 512], F32, tag="pps", name="psS")
                                nc.tensor.matmul(psS,
                                                 lhsT=kT_full[i][base:base+64, JOF[b]*128:JOF[b]*128+128],
                                                 rhs=qTk[i][base:base+64, qb*512:(qb+1)*512],
                                                 start=True, stop=True)
                                if msk is not None:
                                    mb_ = msk[:, gi*128:(gi+1)*128].rearrange(
                                        "p (q o) -> p q o", o=1).to_broadcast((128, 128, 4))
                                    nc.vector.tensor_add(
                                        out=psS.rearrange("p (q o) -> p q o", o=4),
                                        in0=psS.rearrange("p (q o) -> p q o", o=4), in1=mb_)
                                pt = atp.tile([128, 512], F16, tag="pt", name="pt", bufs=4)
                                nc.scalar.activation(out=pt, in_=psS, func=AF.Exp, scale=SCALE)
                                pts.append((b, pt))
                        psO = ppo.tile([128, 512], F32, tag="ppo", name="psO")
                        for bi, (b, pt) in enumerate(pts):
                            nc.tensor.matmul(psO,
                                             lhsT=v_full[:, JOF[b], kvh, :],
                                             rhs=pt,
                                             start=(bi == 0), stop=(bi == nblk - 1))
                        rec = sm.tile([64, 512], F32, tag="rec", name="rec", bufs=2)
                        nc.vector.reciprocal(out=rec, in_=psO[64:128, :])
                        nc.vector.tensor_mul(out=oTk[i][base:base+64, qb*512:(qb+1)*512],
                                             in0=psO[0:64, :], in1=rec)
            # ---- O proj ----
            wot = wp.tile([128, 8, 1024], F16, tag="wot", name="wot", bufs=1)
            dma_a(out=wot, in_=P["wo"][l])
            for d in range(8):
                pso = ppm.tile([128, 256], F32, tag="pm", name="pso")
                for ft in range(8):
                    nc.tensor.matmul(pso, lhsT=wot[:, ft, d*128:(d+1)*128],
                                     rhs=oTk[ft // 4][:, ft % 4::4],
                                     start=(ft == 0), stop=(ft == 7))
                nc.vector.tensor_add(out=xT[d], in0=xT[d], in1=pso)
            # ---- FFN ----
            if SKIP_FFN:
                continue
            h2 = rmsnorm(P["n2h"][l])
            yT = []
            FFN_CHUNKS = [(0, 768), (768, 768), (1536, 768), (2304, 512)]
            for (h0, hwid) in FFN_CHUNKS:
                w1t = wp.tile([128, 8, 768], F16, tag="w1t", name="w1t")
                w3t = wp.tile([128, 8, 768], F16, tag="w3t", name="w3t")
                dma(out=w1t[:, :, 0:hwid], in_=P["w1"][l][:, :, h0:h0+hwid])
                dma_a(out=w3t[:, :, 0:hwid], in_=P["w3"][l][:, :, h0:h0+hwid])
                for hj in range(0, hwid, 128):
                    psu = ppm.tile([128, 256], F32, tag="pm", name="psu")
                    psg = ppm.tile([128, 256], F32, tag="pm", name="psg")
                    for d in range(8):
                        nc.tensor.matmul(psu, lhsT=w1t[:, d, hj:hj+128], rhs=h2[d],
                                         start=(d == 0), stop=(d == 7))
                    for d in range(8):
                        nc.tensor.matmul(psg, lhsT=w3t[:, d, hj:hj+128], rhs=h2[d],
                                         start=(d == 0), stop=(d == 7))
                    su = act.tile([128, 256], F32, tag="su", name="su")
                    nc.scalar.activation(out=su, in_=psu, func=AF.Silu)
                    y = act.tile([128, 256], F16, tag=f"yT{(h0+hj)//128}", name=f"yT{(h0+hj)//128}", bufs=1)
                    nc.vector.tensor_mul(out=y, in0=su, in1=psg)
                    yT.append(y)
            for dpair in range(4):
                w2t = wp.tile([128, 22, 256], F16, tag="w2t", name="w2t")
                dma(out=w2t, in_=P["w2"][l][:, :, dpair*256:(dpair+1)*256])
                for dh in range(2):
                    d = dpair * 2 + dh
                    ps2 = ppm.tile([128, 256], F32, tag="pm", name="ps2")
                    for hc in range(22):
                        nc.tensor.matmul(ps2, lhsT=w2t[:, hc, dh*128:(dh+1)*128],
                                         rhs=yT[hc],
                                         start=(hc == 0), stop=(hc == 21))
                    nc.vector.tensor_add(out=xT[d], in0=xT[d], in1=ps2)
        # ---- final ----
        xf = rmsnorm(P["nfh"][:, :])
        for d in range(8):
            dma_g(out=xf_in[:, d*256:(d+1)*256], in_=xf[d])
        if NO_COLL:
            for r_ in range(8):
                dma_g(out=xf_out[r_*128:(r_+1)*128, :], in_=xf_in[:, :])
        else:
            nc.gpsimd.collective_compute(
                "AllGather", mybir.AluOpType.bypass, replica_groups=RG_ALL,
                ins=[xf_in.ap()], outs=[xf_out.ap()])
        xfT = []
        XF_TAGS = [("wkt", 1), ("wvt", 1), ("w1t", 2), ("w1t", 2),
                   ("w3t", 2), ("w3t", 2), ("w2t", 2), ("w2t", 2)]
        for r in range(8):
            xt = wp.tile([128, 8, 256], F16, tag=XF_TAGS[r][0], bufs=XF_TAGS[r][1], name=f"xfT{r}")
            dma_g(out=xt, in_=xf_out[r*128:(r+1)*128, :].rearrange("p (d t) -> p d t", t=256))
            xfT.append(xt)
        for vc in range(8 if not SKIP_LM else 0):
            embt = wp.tile([128, 8, 500], F16, tag=("wqt", "wot")[vc % 2], bufs=1, name="embt")
            dma_a(out=embt, in_=P["embT"][:, :, vc*500:(vc+1)*500])
            for r in range(8):
                lg = act.tile([128, 2, 500], F16, tag="lg", name="lg", bufs=2)
                for tch in range(2):
                    psl = pps.tile([128, 500], F32, tag="pps", name="psl")
                    for d in range(8):
                        nc.tensor.matmul(psl, lhsT=xfT[r][:, d, tch*128:(tch+1)*128],
                                         rhs=embt[:, d, :], start=(d == 0), stop=(d == 7))
                    if (r + tch) % 2 == 0:
                        nc.vector.tensor_copy(out=lg[:, tch, :], in_=psl)
                    else:
                        nc.scalar.activation(out=lg[:, tch, :], in_=psl, func=AF.Copy)
                if not (HALF_LGWR and vc % 2 == 1):
                    dma(out=logits[r*256:(r+1)*256, vc*500:(vc+1)*500].rearrange("(c p) v -> p c v", p=128),
                        in_=lg)
    nc.compile()
    return nc

_NC_CACHE = {}
def _get_nc():
    if "nc" not in _NC_CACHE:
        _NC_CACHE["nc"] = _build_nc()
    return _NC_CACHE["nc"]

def kernel(**inputs):
    from concourse.bass_utils import run_bass_kernel_spmd
    nc = _get_nc()
    in_maps = _host_prep(**inputs)
    res = run_bass_kernel_spmd(nc, in_maps, list(range(NCORES)))
    return _assemble(res.results)



# revision 25
# speedup vs baseline: 1.0034x; 1.0034x over previous
import numpy as np, sys, os
sys.path.insert(0, "/opt/trn_rl_repo")
NO_COLL = os.environ.get("BASS_NO_COLL", "") == "1"
NLAYERS = int(os.environ.get("BASS_NLAYERS", "8"))
SKIP_ATTN = os.environ.get("BASS_SKIP_ATTN", "") == "1"
SKIP_FFN = os.environ.get("BASS_SKIP_FFN", "") == "1"
SKIP_LM = os.environ.get("BASS_SKIP_LM", "") == "1"
HALF_LGWR = os.environ.get("BASS_HALF_LGWR", "") == "1"
SIM_SILU = os.environ.get("BASS_SIM_SILU", "") == "1"   # CoreSim lacks AF.Silu
NO_FILL = os.environ.get("BASS_NO_FILL", "") == "1"
NO_APPROX = os.environ.get("BASS_NO_APPROX", "") == "1"

V, D, L = 32000, 1024, 8
HQ, HKV, HD = 16, 4, 64
H = 2752
HP = 2816                      # H padded to 22*128
B, S = 2, 1024
WINDOW, GEVERY = 256, 4
EPS, BASE = 1e-6, 10000.0
NCORES = 8
T = 256
VSH = V // NCORES
NEG = -30000.0
SCALE = 1.0 / 8.0

# kv gather order: col-chunk j holds absolute block PI[j]
PI = [0, 7, 1, 6, 2, 5, 3, 4]
JOF = [PI.index(b) for b in range(8)]   # absolute block -> gathered chunk
# head -> (qT tile, base) with base == (kvh%2)*64 so lhsT/rhs partitions align
APAR = [0, 1, 2, 3, 8, 9, 10, 11]       # kvh even -> base 0
BPAR = [4, 5, 6, 7, 12, 13, 14, 15]     # kvh odd  -> base 64
TILE = {}
for t in range(8):
    TILE[APAR[t]] = t
    TILE[BPAR[t]] = t
QPERM = []                               # head order within wq/wo layouts
for t in range(8):
    QPERM += [APAR[t], BPAR[t]]
ROPE_PERM = [2 * f for f in range(32)] + [2 * f + 1 for f in range(32)]

def _core_blocks(c):
    cp = c % 4
    return cp, 7 - cp

def _inv_freq():
    return 1.0 / (BASE ** (np.arange(0, HD, 2, dtype=np.float64) / HD))

def _host_masks(c):
    """Transposed block-masks [k_in_block, (block, q)] in gather-group layout."""
    bA, bB = _core_blocks(c)
    p = np.arange(128)
    def mk(qblk, blocks, local):
        cols = []
        for b in blocks:
            k = (b * 128 + p)[:, None]
            q = (qblk * 128 + p)[None, :]
            valid = k <= q
            if local:
                valid &= (q - k) < WINDOW
            cols.append(np.where(valid, 0.0, NEG))
        return np.concatenate(cols, axis=1).astype(np.float16)
    return (mk(bA, [0, 1, 2, 3], False),       # mAg [128,512]
            mk(bB, [4, 5, 6, 7], False),       # mBg [128,512] (blocks 0-3 maskless)
            mk(bA, [0, 1, 2, 3], True),        # mAl [128,512]
            mk(bB, [2, 3, 4, 5, 6, 7], True))  # mBl [128,768]

def _host_prep(idx, emb, Wq, Wk, Wv, Wo, w1, w3, w2, n1, n2, nf):
    idx = np.asarray(idx)
    emb = np.asarray(emb, dtype=np.float32)
    Wq = np.asarray(Wq, dtype=np.float32); Wk = np.asarray(Wk, dtype=np.float32)
    Wv = np.asarray(Wv, dtype=np.float32); Wo = np.asarray(Wo, dtype=np.float32)
    w1 = np.asarray(w1, dtype=np.float32); w3 = np.asarray(w3, dtype=np.float32)
    w2 = np.asarray(w2, dtype=np.float32)
    invf = _inv_freq()

    # wq: [D,1024] -> heads in QPERM order, rope-permuted cols -> [128, 8, 1024]
    wqv = Wq.reshape(L, D, HQ, HD)[:, :, QPERM][:, :, :, ROPE_PERM]
    wqh = np.ascontiguousarray(
        wqv.reshape(L, 8, 128, HQ * HD).transpose(0, 2, 1, 3).astype(np.float16))
    # wk: natural kvh order, rope-permuted cols
    wkv = Wk.reshape(L, D, HKV, HD)[:, :, :, ROPE_PERM]
    wkh = np.ascontiguousarray(
        wkv.reshape(L, 8, 128, HKV * HD).transpose(0, 2, 1, 3).astype(np.float16))
    wvh = np.ascontiguousarray(
        Wv.reshape(L, 8, 128, HKV * HD).transpose(0, 2, 1, 3).astype(np.float16))
    # wo rows permuted to QPERM head-block order (natural within head)
    wov = Wo.reshape(L, HQ, HD, D)[:, QPERM]
    woh = np.ascontiguousarray(
        wov.reshape(L, 8, 128, D).transpose(0, 2, 1, 3).astype(np.float16))
    # FFN padded to HP
    w1p = np.zeros((L, D, HP), np.float16); w1p[:, :, :H] = w1
    w3p = np.zeros((L, D, HP), np.float16); w3p[:, :, :H] = w3
    w2p = np.zeros((L, HP, D), np.float16); w2p[:, :H, :] = w2
    w1h = np.ascontiguousarray(w1p.reshape(L, 8, 128, HP).transpose(0, 2, 1, 3))
    w3h = np.ascontiguousarray(w3p.reshape(L, 8, 128, HP).transpose(0, 2, 1, 3))
    w2h = np.ascontiguousarray(w2p.reshape(L, 22, 128, D).transpose(0, 2, 1, 3))

    n1h = np.ascontiguousarray(np.asarray(n1).reshape(L, 8, 128).transpose(0, 2, 1)).astype(np.float32)
    n2h = np.ascontiguousarray(np.asarray(n2).reshape(L, 8, 128).transpose(0, 2, 1)).astype(np.float32)
    nfh = np.ascontiguousarray(np.asarray(nf).reshape(8, 128).T).astype(np.float32)

    in_maps = []
    for c in range(NCORES):
        s = c // 4
        bA, bB = _core_blocks(c)
        tok = np.concatenate([idx[s, bA*128:(bA+1)*128], idx[s, bB*128:(bB+1)*128]])
        x0T = np.ascontiguousarray(emb[tok].T)
        pos = np.concatenate([bA*128 + np.arange(128), bB*128 + np.arange(128)])
        ang = invf[:, None] * pos[None, :].astype(np.float64)       # [32, 256]
        cos32 = np.cos(ang).astype(np.float32)
        sin32 = np.sin(ang).astype(np.float32)
        cs = np.concatenate([cos32, cos32, cos32, cos32], axis=0)   # [128,256]
        sc = np.concatenate([-sin32, sin32, -sin32, sin32], axis=0)
        mAg, mBg, mAl, mBl = _host_masks(c)
        embT = np.ascontiguousarray(
            emb[c*VSH:(c+1)*VSH].T.astype(np.float16).reshape(8, 128, VSH).transpose(1, 0, 2))
        in_maps.append({
            "x0T": x0T,
            "wq": wqh,
            "wk": wkh, "wv": wvh, "wo": woh,
            "w1": w1h, "w3": w3h, "w2": w2h,
            "n1h": n1h, "n2h": n2h, "nfh": nfh,
            "cs": cs, "sc": sc,
            "mAg": mAg, "mBg": mBg, "mAl": mAl, "mBl": mBl,
            "embT": embT,
        })
    return in_maps

def _unperm_rows():
    perm = np.zeros(2048, dtype=np.int64)
    for r in range(2048):
        rr, rem = divmod(r, 256)
        slot, p = divmod(rem, 128)
        samp = rr // 4
        bA, bB = _core_blocks(rr)
        blk = bA if slot == 0 else bB
        perm[r] = samp * S + blk * 128 + p
    inv = np.zeros(2048, dtype=np.int64)
    inv[perm] = np.arange(2048)
    return inv

def _assemble(outs):
    full = np.empty((2048, V), dtype=np.float32)
    for c in range(NCORES):
        full[:, c*VSH:(c+1)*VSH] = outs[c]["logits"].astype(np.float32)
    inv = _unperm_rows()
    return full[inv].reshape(B, S, V)

def _build_nc():
    import concourse.bass as bass
    import concourse.bacc as bacc
    import concourse.mybir as mybir
    from concourse.tile import TileContext
    F32, F16, F32R = mybir.dt.float32, mybir.dt.float16, mybir.dt.float32r
    AF = mybir.ActivationFunctionType
    ALU = mybir.AluOpType

    nc = bacc.Bacc("TRN2", target_bir_lowering=False, debug=False, num_devices=NCORES)
    P = {}
    def inp(name, shape, dt=F16):
        P[name] = nc.declare_dram_parameter(name, list(shape), dt, isOutput=False)
    inp("x0T", (D, T), F32)
    inp("wq", (L, 128, 8, 1024)); inp("wk", (L, 128, 8, 256)); inp("wv", (L, 128, 8, 256))
    inp("wo", (L, 128, 8, 1024))
    inp("w1", (L, 128, 8, HP)); inp("w3", (L, 128, 8, HP)); inp("w2", (L, 128, 22, 1024))
    inp("n1h", (L, 128, 8), F32); inp("n2h", (L, 128, 8), F32); inp("nfh", (128, 8), F32)
    inp("cs", (128, 256), F32); inp("sc", (128, 256), F32)
    inp("mAg", (128, 512)); inp("mBg", (128, 512))
    inp("mAl", (128, 512)); inp("mBl", (128, 768))
    inp("embT", (128, 8, VSH))
    logits = nc.declare_dram_parameter("logits", [2048, VSH], F16, isOutput=True)

    kv_in  = [nc.dram_tensor(f"kv_in{l}",  [4, 128, 256], F16) for l in range(L)]
    kv_out = [nc.dram_tensor(f"kv_out{l}", [16, 128, 256], F16) for l in range(L)]
    warm_in = nc.dram_tensor("warm_in", [1, 4], F16)
    warm_out = nc.dram_tensor("warm_out", [8, 4], F16)
    xf_in  = [nc.dram_tensor(f"xf_in{h_}", [128, 1024], F16) for h_ in range(2)]
    xf_out = [nc.dram_tensor(f"xf_out{h_}", [NCORES * 128, 1024], F16, addr_space="Shared")
              for h_ in range(2)]
    RG_KV = [[0, 1, 2, 3], [4, 5, 6, 7]]
    RG_ALL = [list(range(NCORES))]

    with TileContext(nc) as tc:
      with tc.tile_pool(name="pers", bufs=1) as pers, \
           tc.tile_pool(name="wpool", bufs=2) as wp, \
           tc.tile_pool(name="act", bufs=2) as act, \
           tc.tile_pool(name="attn", bufs=2) as atp, \
           tc.tile_pool(name="small", bufs=4) as sm, \
           tc.tile_pool(name="ppffn", bufs=4, space="PSUM") as ppm, \
           tc.tile_pool(name="pps", bufs=2, space="PSUM") as pps, \
           tc.tile_pool(name="ppo", bufs=2, space="PSUM") as ppo:

        dma = nc.sync.dma_start
        dma_a = nc.scalar.dma_start
        dma_g = nc.gpsimd.dma_start
        # warmup collective: absorbs the comm-bootstrap rendezvous (~80us)
        # while the initial input/weight DMAs and early compute proceed.
        if not NO_COLL:
            nc.gpsimd.collective_compute(
                "AllGather", mybir.AluOpType.bypass, replica_groups=RG_ALL,
                ins=[warm_in.ap()], outs=[warm_out.ap()])
        xT = [pers.tile([128, T], F32, tag=f"xT{d}", name=f"xT{d}") for d in range(8)]
        for d in range(8):
            dma_g(out=xT[d], in_=P["x0T"][d*128:(d+1)*128, :])

        # prefetch layer-l attention weights; issued one stage early so the
        # DMA runs during the previous layer's FFN.
        wtiles = {}
        def load_attn_weights(l):
            wkt = wp.tile([128, 8, 256], F16, tag="wkt", name=f"wkt{l}", bufs=1)
            wvt = wp.tile([128, 8, 256], F16, tag="wvt", name=f"wvt{l}", bufs=1)
            wqt = wp.tile([128, 8, 1024], F16, tag="wqt", name=f"wqt{l}", bufs=1)
            wot = wp.tile([128, 8, 1024], F16, tag="wot", name=f"wot{l}", bufs=1)
            dma(out=wkt, in_=P["wk"][l])
            dma(out=wvt, in_=P["wv"][l])
            dma(out=wqt, in_=P["wq"][l])
            dma_a(out=wot, in_=P["wo"][l])
            wtiles[l] = (wkt, wvt, wqt, wot)
        cs = pers.tile([128, 256], F32, tag="cs", name="cs"); dma_g(out=cs, in_=P["cs"][:, :])
        sc = pers.tile([128, 256], F32, tag="sc", name="sc"); dma_g(out=sc, in_=P["sc"][:, :])
        mAg = pers.tile([128, 512], F16, tag="mAg", name="mAg"); dma_g(out=mAg, in_=P["mAg"][:, :])
        mBg = pers.tile([128, 512], F16, tag="mBg", name="mBg"); dma_g(out=mBg, in_=P["mBg"][:, :])
        mAl = pers.tile([128, 512], F16, tag="mAl", name="mAl"); dma_g(out=mAl, in_=P["mAl"][:, :])
        mBl = pers.tile([128, 768], F16, tag="mBl", name="mBl"); dma_g(out=mBl, in_=P["mBl"][:, :])
        # persistent V tiles (parity double-buffer); ones half memset once
        v_full2 = [pers.tile([128, 8, 4, 128], F16, tag=f"vfull{p_}", name=f"vfull{p_}")
                   for p_ in range(2)]
        for p_ in range(2):
            nc.gpsimd.memset(v_full2[p_][:, :, :, 64:128], 1.0)
        onesf = pers.tile([128, 1], F32, tag="onesf", name="onesf")
        nc.vector.memset(onesf, 1.0)
        ones = pers.tile([128, 1], F32R, tag="ones", name="ones")
        nc.vector.tensor_copy(out=ones, in_=onesf)
        ones1f = pers.tile([1, 128], F32, tag="ones1f", name="ones1f")
        nc.vector.memset(ones1f, 1.0)
        ones1r = pers.tile([1, 128], F32R, tag="ones1r", name="ones1r")
        nc.vector.tensor_copy(out=ones1r, in_=ones1f)
        epst = pers.tile([1, 1], F32, tag="epst", name="epst")
        nc.vector.memset(epst, EPS)

        def rmsnorm(nw_dram):
            nw = sm.tile([128, 8], F32, tag="nw", name="nw")
            dma_g(out=nw, in_=nw_dram)
            ss = ppm.tile([1, T], F32, tag="pm", name="ss")
            for d in range(8):
                x2 = act.tile([128, T], F32R, tag="x2", name="x2")
                nc.vector.tensor_mul(out=x2, in0=xT[d], in1=xT[d])
                nc.tensor.matmul(ss, lhsT=ones, rhs=x2, start=(d == 0), stop=(d == 7))
            rrow = sm.tile([1, T], F32, tag="rrow", name="rrow", bufs=2)
            nc.scalar.activation(out=rrow, in_=ss, func=AF.Sqrt, scale=1.0/D, bias=epst[0:1, 0:1])
            rrec = sm.tile([1, T], F32R, tag="rrec", name="rrec", bufs=2)
            if NO_APPROX:
                with nc.allow_low_precision(reason="rsqrt broadcast row, f32r for PE outer-product"):
                    nc.vector.reciprocal(out=rrec, in_=rrow)
            else:
                rraw = sm.tile([1, T], F32, tag="rraw", name="rraw", bufs=2)
                nc.vector.reciprocal_approx_fast(out=rraw, in_=rrow)
                with nc.allow_low_precision(reason="rsqrt broadcast row, f32r for PE outer-product"):
                    nc.vector.tensor_copy(out=rrec, in_=rraw)
            rb = ppm.tile([128, T], F32, tag="pm", name="rb")
            nc.tensor.matmul(rb, lhsT=ones1r, rhs=rrec, start=True, stop=True)
            out = []
            for d in range(8):
                h = act.tile([128, T], F16, tag=f"hT{d}", name=f"hT{d}", bufs=1)
                nc.vector.scalar_tensor_tensor(out=h, in0=xT[d], scalar=nw[:, d:d+1],
                                               in1=rb, op0=ALU.mult, op1=ALU.mult)
                out.append(h)
            return out

        def rope_tile(ps, outt):
            """qt = ps*COS + swap32(ps)*SS;  rows = [e0,o0,e1,o1] 32-blocks."""
            swp = sm.tile([128, 256], F32, tag="ropeswp", name="ropeswp", bufs=2)
            for j in range(4):
                sj = j ^ 1
                nc.scalar.activation(out=swp[j*32:(j+1)*32, :],
                                     in_=ps[sj*32:(sj+1)*32, :], func=AF.Copy)
            u = sm.tile([128, 256], F32, tag="ropeu", name="ropeu", bufs=2)
            t = sm.tile([128, 256], F32, tag="ropet", name="ropet", bufs=2)
            nc.vector.tensor_mul(out=u, in0=ps, in1=cs)
            nc.vector.tensor_mul(out=t, in0=swp, in1=sc)
            nc.vector.tensor_add(out=outt, in0=u, in1=t)

        def pe_filler(n):
            """Dependency-free fp32 matmuls: execute during collective waits,
            keeping the PE HAM clock warm; result is discarded."""
            fd = ppm.tile([128, 256], F32, tag="pm", name="fill")
            for k_ in range(n):
                nc.tensor.matmul(fd, lhsT=cs[:, 0:128], rhs=sc,
                                 start=(k_ == 0), stop=(k_ == n - 1))

        FFN_CHUNKS = [(0, 768), (768, 768), (1536, 768), (2304, 512)]
        ffn_tiles = {}
        def ffn_issue_chunk(l, c):
            h0, hwid = FFN_CHUNKS[c]
            w1t = wp.tile([128, 8, 768], F16, tag="w1t", name=f"w1t{l}_{c}")
            w3t = wp.tile([128, 8, 768], F16, tag="w3t", name=f"w3t{l}_{c}")
            dma(out=w1t[:, :, 0:hwid], in_=P["w1"][l][:, :, h0:h0+hwid])
            dma_a(out=w3t[:, :, 0:hwid], in_=P["w3"][l][:, :, h0:h0+hwid])
            ffn_tiles[(l, c)] = (w1t, w3t)
        w2_tiles = {}
        def w2_issue(l, hc):
            w2c = wp.tile([128, 512], F16, tag="w2c", name=f"w2c{l}_{hc}", bufs=3)
            dma_g(out=w2c, in_=P["w2"][l][:, hc, 0:512])
            w2_tiles[(l, hc)] = w2c
        w2hi_tiles = {}
        def w2hi_issue(l, hc):
            w2h_ = wp.tile([128, 512], F16, tag="w2h", name=f"w2h{l}_{hc}", bufs=4)
            dma(out=w2h_, in_=P["w2"][l][:, hc, 512:1024])
            w2hi_tiles[(l, hc)] = w2h_

        load_attn_weights(0)
        for l in range(NLAYERS):
            is_global = ((l + 1) % GEVERY) == 0
            wkt, wvt, wqt, wot = wtiles.pop(l)
            h1 = rmsnorm(P["n1h"][l])
            # ---- K^T (direct), V ----
            for i in range(2):
                psk = ppm.tile([128, 256], F32, tag="pm", name="psk")
                for d in range(8):
                    nc.tensor.matmul(psk, lhsT=wkt[:, d, i*128:(i+1)*128], rhs=h1[d],
                                     start=(d == 0), stop=(d == 7))
                kt = atp.tile([128, 256], F16, tag=f"ktc{i}", name=f"ktc{i}")
                rope_tile(psk, kt)
                dma_g(out=kv_in[l][i], in_=kt)
            for t2_ in range(2):
                psv = ppm.tile([128, 256], F32, tag="pm", name="psv")
                for d in range(8):
                    nc.tensor.matmul(psv, lhsT=h1[d][:, t2_*128:(t2_+1)*128], rhs=wvt[:, d, :],
                                     start=(d == 0), stop=(d == 7))
                vt = atp.tile([128, 256], F16, tag=f"vtok{t2_}", name=f"vtok{t2_}")
                nc.vector.tensor_copy(out=vt, in_=psv)
                dma_g(out=kv_in[l][2 + t2_], in_=vt)
            if NO_COLL:
                for r_ in range(4):
                    dma_g(out=kv_out[l][r_*4:(r_+1)*4], in_=kv_in[l][:])
            else:
                nc.gpsimd.collective_compute(
                    "AllGather", mybir.AluOpType.bypass, replica_groups=RG_KV,
                    ins=[kv_in[l].ap()], outs=[kv_out[l].ap()])
            # ---- Q^T (direct) ----
            qTk = [act.tile([128, 1024], F16, tag=f"qTk{i}", name=f"qTk{i}", bufs=1)
                   for i in range(2)]
            for t in range(8):
                i, j = t // 4, t % 4
                psq = ppm.tile([128, 256], F32, tag="pm", name="psq")
                for d in range(8):
                    nc.tensor.matmul(psq, lhsT=wqt[:, d, t*128:(t+1)*128], rhs=h1[d],
                                     start=(d == 0), stop=(d == 7))
                rope_tile(psq, qTk[i][:, j::4])
            # prefetch FFN weights while attention runs
            if not SKIP_FFN:
                ffn_issue_chunk(l, 0)
                ffn_issue_chunk(l, 1)
                for hc_ in range(3):
                    w2_issue(l, hc_)
            # ---- gather K/V from collective ----
            kT_full = [atp.tile([128, 1024], F16, tag=f"kTf{i}", name=f"kTf{i}", bufs=1) for i in range(2)]
            for i in range(2):
                dma_g(out=kT_full[i].rearrange("p (r t) -> p r t", r=4),
                    in_=kv_out[l][i::4].rearrange("r p t -> p r t"))
            v_full = v_full2[l % 2]
            for sl in range(2):
                for r in range(4):
                    dma_g(out=v_full[:, 2*r+sl, :, 0:64],
                        in_=kv_out[l][r*4+2+sl].rearrange("p (f h) -> p f h", h=64))
            if not NO_FILL:
                pe_filler(14)
            # ---- attention (4 q heads batched per kv head; sl pairs
            # interleaved so row-groups 0-63/64-127 run concurrently) ----
            oTk = [act.tile([128, 1024], F16, tag=f"oTk{i}", name=f"oTk{i}", bufs=1)
                   for i in range(2)]
            for i in range(2 if not SKIP_ATTN else 0):
                for qb in range(2):
                    if qb == 0:
                        groups = [([0, 1, 2, 3], mAg if is_global else mAl)]
                    elif is_global:
                        groups = [([0, 1, 2, 3], None), ([4, 5, 6, 7], mBg)]
                    else:
                        groups = [([2, 3, 4, 5], mBl[:, 0:512]), ([6, 7], mBl[:, 512:768])]
                    nblk = sum(len(g[0]) for g in groups)
                    pts = {0: [], 1: []}
                    for (blocks, msk) in groups:
                        for gi, b in enumerate(blocks):
                            for sl in range(2):
                                base = sl * 64
                                psS = pps.tile([128, 512], F32, tag="pps", name="psS")
                                nc.tensor.matmul(psS,
                                                 lhsT=kT_full[i][base:base+64, JOF[b]*128:JOF[b]*128+128],
                                                 rhs=qTk[i][base:base+64, qb*512:(qb+1)*512],
                                                 start=True, stop=True)
                                if msk is not None:
                                    mb_ = msk[:, gi*128:(gi+1)*128].rearrange(
                                        "p (q o) -> p q o", o=1).to_broadcast((128, 128, 4))
                                    nc.vector.tensor_add(
                                        out=psS.rearrange("p (q o) -> p q o", o=4),
                                        in0=psS.rearrange("p (q o) -> p q o", o=4), in1=mb_)
                                pt = atp.tile([128, 512], F16, tag="pt", name="pt", bufs=4)
                                nc.scalar.activation(out=pt, in_=psS, func=AF.Exp, scale=SCALE)
                                pts[sl].append((b, pt))
                    psO = {}
                    for sl in range(2):
                        psO[sl] = ppo.tile([128, 512], F32, tag="ppo", name="psO")
                    for bi in range(nblk):
                        for sl in range(2):
                            b, pt = pts[sl][bi]
                            nc.tensor.matmul(psO[sl],
                                             lhsT=v_full[:, JOF[b], 2*i+sl, :],
                                             rhs=pt,
                                             start=(bi == 0), stop=(bi == nblk - 1))
                    for sl in range(2):
                        base = sl * 64
                        rec = sm.tile([64, 512], F32, tag="rec", name="rec", bufs=2)
                        if NO_APPROX:
                            nc.vector.reciprocal(out=rec, in_=psO[sl][64:128, :])
                        else:
                            nc.vector.reciprocal_approx_fast(out=rec, in_=psO[sl][64:128, :])
                        nc.vector.tensor_mul(out=oTk[i][base:base+64, qb*512:(qb+1)*512],
                                             in0=psO[sl][0:64, :], in1=rec)
            # ---- O proj ----
            for d in range(8):
                pso = ppm.tile([128, 256], F32, tag="pm", name="pso")
                for ft in range(8):
                    nc.tensor.matmul(pso, lhsT=wot[:, ft, d*128:(d+1)*128],
                                     rhs=oTk[ft // 4][:, ft % 4::4],
                                     start=(ft == 0), stop=(ft == 7))
                nc.vector.tensor_add(out=xT[d], in0=xT[d], in1=pso)
            # prefetch next layer's attention weights during this FFN
            if l + 1 < NLAYERS:
                load_attn_weights(l + 1)
            # ---- FFN (producer/consumer interleaved per 128-col chunk) ----
            if SKIP_FFN:
                continue
            h2 = rmsnorm(P["n2h"][l])
            # phase-1 accumulators for d 0-3 (one chain per PSUM bank; a bank
            # cannot host two start/stop chains - start zeroes the whole row)
            banks = [pps.tile([128, 256], F32, tag="pps", name=f"fb{k}") for k in range(2)] \
                  + [ppo.tile([128, 256], F32, tag="ppo", name=f"fb{k+2}") for k in range(2)]
            yT = {}
            pend = []          # hc awaiting phase-1 w2 consumption
            def w2_consume():
                hc = pend.pop(0)
                w2c = w2_tiles.pop((l, hc))
                for d in range(4):
                    nc.tensor.matmul(banks[d],
                                     lhsT=w2c[:, d*128:(d+1)*128], rhs=yT[hc],
                                     start=(hc == 0), stop=(hc == 21))
            for c, (h0, hwid) in enumerate(FFN_CHUNKS):
                if c + 2 < len(FFN_CHUNKS):
                    ffn_issue_chunk(l, c + 2)
                w1t, w3t = ffn_tiles.pop((l, c))
                for hj in range(0, hwid, 128):
                    hc = (h0 + hj) // 128
                    if hc + 3 < 22:
                        w2_issue(l, hc + 3)
                    psu = ppm.tile([128, 256], F32, tag="pm", name="psu")
                    psg = ppm.tile([128, 256], F32, tag="pm", name="psg")
                    for d in range(8):
                        nc.tensor.matmul(psu, lhsT=w1t[:, d, hj:hj+128], rhs=h2[d],
                                         start=(d == 0), stop=(d == 7))
                    for d in range(8):
                        nc.tensor.matmul(psg, lhsT=w3t[:, d, hj:hj+128], rhs=h2[d],
                                         start=(d == 0), stop=(d == 7))
                    su = act.tile([128, 256], F32, tag="su", name="su")
                    y = act.tile([128, 256], F16, tag=f"yT{hc}", name=f"yT{hc}", bufs=1)
                    if SIM_SILU:
                        nc.scalar.activation(out=su, in_=psu, func=AF.Sigmoid)
                        nc.vector.tensor_mul(out=su, in0=su, in1=psu)
                        nc.vector.tensor_mul(out=y, in0=su, in1=psg)
                    else:
                        nc.scalar.activation(out=su, in_=psu, func=AF.Silu)
                        nc.vector.tensor_mul(out=y, in0=su, in1=psg)
                    yT[hc] = y
                    pend.append(hc)
                    if len(pend) > 1:
                        w2_consume()
            while pend:
                w2_consume()
            for d in range(4):
                nc.vector.tensor_add(out=xT[d], in0=xT[d], in1=banks[d])
            # phase 2: d 4-7 over all 22 chunks (hi half of w2 streamed on sync q)
            banks2 = [pps.tile([128, 256], F32, tag="pps", name=f"fb2{k}") for k in range(2)] \
                   + [ppo.tile([128, 256], F32, tag="ppo", name=f"fb2{k+2}") for k in range(2)]
            for hc_ in range(4):
                w2hi_issue(l, hc_)
            for hc in range(22):
                if hc + 4 < 22:
                    w2hi_issue(l, hc + 4)
                w2h_ = w2hi_tiles.pop((l, hc))
                for dh in range(4):
                    nc.tensor.matmul(banks2[dh],
                                     lhsT=w2h_[:, dh*128:(dh+1)*128], rhs=yT[hc],
                                     start=(hc == 0), stop=(hc == 21))
            for dh in range(4):
                nc.vector.tensor_add(out=xT[4+dh], in0=xT[4+dh], in1=banks2[dh])
        # ---- final ----
        xf = rmsnorm(P["nfh"][:, :])
        for h_ in range(2):
            for dd in range(4):
                d = h_ * 4 + dd
                dma_g(out=xf_in[h_][:, dd*256:(dd+1)*256], in_=xf[d])
            if NO_COLL:
                for r_ in range(8):
                    dma_g(out=xf_out[h_][r_*128:(r_+1)*128, :], in_=xf_in[h_][:, :])
            else:
                nc.gpsimd.collective_compute(
                    "AllGather", mybir.AluOpType.bypass, replica_groups=RG_ALL,
                    ins=[xf_in[h_].ap()], outs=[xf_out[h_].ap()])
        if not NO_FILL:
            pe_filler(56)
        xfT = []
        XF_TAGS = [("wkt", 1), ("wvt", 1), ("w1t", 2), ("w1t", 2),
                   ("w3t", 2), ("w3t", 2), ("w2c", 3), ("w2c", 3)]
        for r in range(8):
            xt = wp.tile([128, 2, 4, 256], F16, tag=XF_TAGS[r][0], bufs=XF_TAGS[r][1], name=f"xfT{r}")
            for h_ in range(2):
                dma_g(out=xt[:, h_], in_=xf_out[h_][r*128:(r+1)*128, :].rearrange("p (d t) -> p d t", t=256))
            xfT.append(xt)
        for vc in range(8 if not SKIP_LM else 0):
            embt = wp.tile([128, 8, 500], F16, tag=("wqt", "wot")[vc % 2], bufs=1, name="embt")
            dma_a(out=embt, in_=P["embT"][:, :, vc*500:(vc+1)*500])
            for r in range(8):
                lg = act.tile([128, 2, 500], F16, tag="lg", name="lg", bufs=2)
                for tch in range(2):
                    psl = pps.tile([128, 500], F32, tag="pps", name="psl")
                    for d in range(8):
                        nc.tensor.matmul(psl, lhsT=xfT[r][:, d // 4, d % 4, tch*128:(tch+1)*128],
                                         rhs=embt[:, d, :], start=(d == 0), stop=(d == 7))
                    if (r + tch) % 2 == 0:
                        nc.vector.tensor_copy(out=lg[:, tch, :], in_=psl)
                    else:
                        nc.scalar.activation(out=lg[:, tch, :], in_=psl, func=AF.Copy)
                if not (HALF_LGWR and vc % 2 == 1):
                    dma(out=logits[r*256:(r+1)*256, vc*500:(vc+1)*500].rearrange("(c p) v -> p c v", p=128),
                        in_=lg)
    nc.compile()
    return nc

_NC_CACHE = {}
def _get_nc():
    if "nc" not in _NC_CACHE:
        _NC_CACHE["nc"] = _build_nc()
    return _NC_CACHE["nc"]

def kernel(**inputs):
    from concourse.bass_utils import run_bass_kernel_spmd
    nc = _get_nc()
    in_maps = _host_prep(**inputs)
    res = run_bass_kernel_spmd(nc, in_maps, list(range(NCORES)))
    return _assemble(res.results)



# revision 42
# speedup vs baseline: 1.0791x; 1.0755x over previous
import numpy as np, sys, os
sys.path.insert(0, "/opt/trn_rl_repo")
NO_COLL = os.environ.get("BASS_NO_COLL", "") == "1"
NLAYERS = int(os.environ.get("BASS_NLAYERS", "8"))
SKIP_ATTN = os.environ.get("BASS_SKIP_ATTN", "") == "1"
SKIP_FFN = os.environ.get("BASS_SKIP_FFN", "") == "1"
SKIP_LM = os.environ.get("BASS_SKIP_LM", "") == "1"
HALF_LGWR = os.environ.get("BASS_HALF_LGWR", "") == "1"
SIM_SILU = os.environ.get("BASS_SIM_SILU", "") == "1"   # CoreSim lacks AF.Silu
NO_FILL = os.environ.get("BASS_NO_FILL", "") == "1"
NO_APPROX = os.environ.get("BASS_NO_APPROX", "") == "1"
APPROX_SBUF = os.environ.get("BASS_APPROX_SBUF", "") == "1"
WARM_COLL = os.environ.get("BASS_WARM_COLL", "") == "1"

V, D, L = 32000, 1024, 8
HQ, HKV, HD = 16, 4, 64
H = 2752
HP = 2816                      # H padded to 22*128
B, S = 2, 1024
WINDOW, GEVERY = 256, 4
EPS, BASE = 1e-6, 10000.0
NCORES = 8
T = 256
VSH = V // NCORES
NEG = -30000.0
SCALE = 1.0 / 8.0

# kv gather order: col-chunk j holds absolute block PI[j]
PI = [0, 7, 1, 6, 2, 5, 3, 4]
JOF = [PI.index(b) for b in range(8)]   # absolute block -> gathered chunk
# head -> (qT tile, base) with base == (kvh%2)*64 so lhsT/rhs partitions align
APAR = [0, 1, 2, 3, 8, 9, 10, 11]       # kvh even -> base 0
BPAR = [4, 5, 6, 7, 12, 13, 14, 15]     # kvh odd  -> base 64
TILE = {}
for t in range(8):
    TILE[APAR[t]] = t
    TILE[BPAR[t]] = t
QPERM = []                               # head order within wq/wo layouts
for t in range(8):
    QPERM += [APAR[t], BPAR[t]]
ROPE_PERM = [2 * f for f in range(32)] + [2 * f + 1 for f in range(32)]

def _core_blocks(c):
    cp = c % 4
    return cp, 7 - cp

def _inv_freq():
    return 1.0 / (BASE ** (np.arange(0, HD, 2, dtype=np.float64) / HD))

def _host_masks(c):
    """Transposed block-masks [k_in_block, (block, q)] in gather-group layout."""
    bA, bB = _core_blocks(c)
    p = np.arange(128)
    def mk(qblk, blocks, local):
        cols = []
        for b in blocks:
            k = (b * 128 + p)[:, None]
            q = (qblk * 128 + p)[None, :]
            valid = k <= q
            if local:
                valid &= (q - k) < WINDOW
            cols.append(np.where(valid, 0.0, NEG))
        return np.concatenate(cols, axis=1).astype(np.float16)
    return (mk(bA, [0, 1, 2, 3], False),       # mAg [128,512]
            mk(bB, [4, 5, 6, 7], False),       # mBg [128,512] (blocks 0-3 maskless)
            mk(bA, [0, 1, 2, 3], True),        # mAl [128,512]
            mk(bB, [2, 3, 4, 5, 6, 7], True))  # mBl [128,768]

def _host_prep(idx, emb, Wq, Wk, Wv, Wo, w1, w3, w2, n1, n2, nf):
    idx = np.asarray(idx)
    emb = np.asarray(emb, dtype=np.float32)
    Wq = np.asarray(Wq, dtype=np.float32); Wk = np.asarray(Wk, dtype=np.float32)
    Wv = np.asarray(Wv, dtype=np.float32); Wo = np.asarray(Wo, dtype=np.float32)
    w1 = np.asarray(w1, dtype=np.float32); w3 = np.asarray(w3, dtype=np.float32)
    w2 = np.asarray(w2, dtype=np.float32)
    invf = _inv_freq()

    # wq: [D,1024] -> heads in QPERM order, rope-permuted cols -> [128, 8, 1024]
    wqv = Wq.reshape(L, D, HQ, HD)[:, :, QPERM][:, :, :, ROPE_PERM]
    wqh = np.ascontiguousarray(
        wqv.reshape(L, 8, 128, HQ * HD).transpose(0, 2, 1, 3).astype(np.float16))
    # wk: natural kvh order, rope-permuted cols
    wkv = Wk.reshape(L, D, HKV, HD)[:, :, :, ROPE_PERM]
    wkh = np.ascontiguousarray(
        wkv.reshape(L, 8, 128, HKV * HD).transpose(0, 2, 1, 3).astype(np.float16))
    # 32-col pair-blocks swapped: produce swap32(q)/swap32(k) directly on PE
    wqsw = np.ascontiguousarray(
        wqh.reshape(L, 128, 8, 16, 2, 32)[:, :, :, :, ::-1, :].reshape(L, 128, 8, 1024))
    wksw = np.ascontiguousarray(
        wkh.reshape(L, 128, 8, 4, 2, 32)[:, :, :, :, ::-1, :].reshape(L, 128, 8, 256))
    wvh = np.ascontiguousarray(
        Wv.reshape(L, 8, 128, HKV * HD).transpose(0, 2, 1, 3).astype(np.float16))
    # wo rows permuted to QPERM head-block order (natural within head)
    wov = Wo.reshape(L, HQ, HD, D)[:, QPERM]
    woh = np.ascontiguousarray(
        wov.reshape(L, 8, 128, D).transpose(0, 2, 1, 3).astype(np.float16))
    # FFN padded to HP
    w1p = np.zeros((L, D, HP), np.float16); w1p[:, :, :H] = w1
    w3p = np.zeros((L, D, HP), np.float16); w3p[:, :, :H] = w3
    w2p = np.zeros((L, HP, D), np.float16); w2p[:, :H, :] = w2
    w1h = np.ascontiguousarray(w1p.reshape(L, 8, 128, HP).transpose(0, 2, 1, 3))
    w3h = np.ascontiguousarray(w3p.reshape(L, 8, 128, HP).transpose(0, 2, 1, 3))
    w2h = np.ascontiguousarray(w2p.reshape(L, 22, 128, D).transpose(0, 2, 1, 3))

    n1h = np.ascontiguousarray(np.asarray(n1).reshape(L, 8, 128).transpose(0, 2, 1)).astype(np.float32)
    n2h = np.ascontiguousarray(np.asarray(n2).reshape(L, 8, 128).transpose(0, 2, 1)).astype(np.float32)
    nfh = np.ascontiguousarray(np.asarray(nf).reshape(8, 128).T).astype(np.float32)

    in_maps = []
    for c in range(NCORES):
        s = c // 4
        bA, bB = _core_blocks(c)
        tok = np.concatenate([idx[s, bA*128:(bA+1)*128], idx[s, bB*128:(bB+1)*128]])
        x0T = np.ascontiguousarray(emb[tok].T)
        pos = np.concatenate([bA*128 + np.arange(128), bB*128 + np.arange(128)])
        ang = invf[:, None] * pos[None, :].astype(np.float64)       # [32, 256]
        cos32 = np.cos(ang).astype(np.float32)
        sin32 = np.sin(ang).astype(np.float32)
        cs = np.concatenate([cos32, cos32, cos32, cos32], axis=0)   # [128,256]
        sc = np.concatenate([-sin32, sin32, -sin32, sin32], axis=0)
        mAg, mBg, mAl, mBl = _host_masks(c)
        embT = np.ascontiguousarray(
            emb[c*VSH:(c+1)*VSH].T.astype(np.float16).reshape(8, 128, VSH).transpose(1, 0, 2))
        in_maps.append({
            "x0T": x0T,
            "wq": wqh, "wqsw": wqsw, "wksw": wksw,
            "wk": wkh, "wv": wvh, "wo": woh,
            "w1": w1h, "w3": w3h, "w2": w2h,
            "n1h": n1h, "n2h": n2h, "nfh": nfh,
            "cs": cs, "sc": sc,
            "mAg": mAg, "mBg": mBg, "mAl": mAl, "mBl": mBl,
            "embT": embT,
        })
    return in_maps

def _unperm_rows():
    perm = np.zeros(2048, dtype=np.int64)
    for r in range(2048):
        rr, rem = divmod(r, 256)
        slot, p = divmod(rem, 128)
        samp = rr // 4
        bA, bB = _core_blocks(rr)
        blk = bA if slot == 0 else bB
        perm[r] = samp * S + blk * 128 + p
    inv = np.zeros(2048, dtype=np.int64)
    inv[perm] = np.arange(2048)
    return inv

def _assemble(outs):
    full = np.empty((2048, V), dtype=np.float32)
    for c in range(NCORES):
        full[:, c*VSH:(c+1)*VSH] = outs[c]["logits"].astype(np.float32)
    inv = _unperm_rows()
    return full[inv].reshape(B, S, V)

def _build_nc():
    import concourse.bass as bass
    import concourse.bacc as bacc
    import concourse.mybir as mybir
    from concourse.tile import TileContext
    F32, F16, F32R = mybir.dt.float32, mybir.dt.float16, mybir.dt.float32r
    AF = mybir.ActivationFunctionType
    ALU = mybir.AluOpType

    nc = bacc.Bacc("TRN2", target_bir_lowering=False, debug=False, num_devices=NCORES)
    P = {}
    def inp(name, shape, dt=F16):
        P[name] = nc.declare_dram_parameter(name, list(shape), dt, isOutput=False)
    inp("x0T", (D, T), F32)
    inp("wq", (L, 128, 8, 1024)); inp("wk", (L, 128, 8, 256)); inp("wv", (L, 128, 8, 256))
    inp("wqsw", (L, 128, 8, 1024)); inp("wksw", (L, 128, 8, 256))
    inp("wo", (L, 128, 8, 1024))
    inp("w1", (L, 128, 8, HP)); inp("w3", (L, 128, 8, HP)); inp("w2", (L, 128, 22, 1024))
    inp("n1h", (L, 128, 8), F32); inp("n2h", (L, 128, 8), F32); inp("nfh", (128, 8), F32)
    inp("cs", (128, 256), F32); inp("sc", (128, 256), F32)
    inp("mAg", (128, 512)); inp("mBg", (128, 512))
    inp("mAl", (128, 512)); inp("mBl", (128, 768))
    inp("embT", (128, 8, VSH))
    logits = nc.declare_dram_parameter("logits", [2048, VSH], F16, isOutput=True)

    kv_in  = [nc.dram_tensor(f"kv_in{l}",  [4, 128, 256], F16) for l in range(L)]
    kv_out = [nc.dram_tensor(f"kv_out{l}", [16, 128, 256], F16) for l in range(L)]
    warm_in = nc.dram_tensor("warm_in", [1, 4], F16)
    warm_out = nc.dram_tensor("warm_out", [8, 4], F16)
    xf_in  = [nc.dram_tensor(f"xf_in{h_}", [128, 1024], F16) for h_ in range(2)]
    xf_out = [nc.dram_tensor(f"xf_out{h_}", [NCORES * 128, 1024], F16, addr_space="Shared")
              for h_ in range(2)]
    RG_KV = [[0, 1, 2, 3], [4, 5, 6, 7]]
    RG_ALL = [list(range(NCORES))]

    with TileContext(nc) as tc:
      with tc.tile_pool(name="pers", bufs=1) as pers, \
           tc.tile_pool(name="wpool", bufs=2) as wp, \
           tc.tile_pool(name="act", bufs=2) as act, \
           tc.tile_pool(name="attn", bufs=2) as atp, \
           tc.tile_pool(name="small", bufs=4) as sm, \
           tc.tile_pool(name="ppffn", bufs=4, space="PSUM") as ppm, \
           tc.tile_pool(name="pps", bufs=2, space="PSUM") as pps, \
           tc.tile_pool(name="ppo", bufs=2, space="PSUM") as ppo:

        dma = nc.sync.dma_start
        dma_a = nc.scalar.dma_start
        dma_g = nc.gpsimd.dma_start
        if WARM_COLL and not NO_COLL:
            nc.gpsimd.collective_compute(
                "AllGather", mybir.AluOpType.bypass, replica_groups=RG_ALL,
                ins=[warm_in.ap()], outs=[warm_out.ap()])
        xT = [pers.tile([128, T], F32, tag=f"xT{d}", name=f"xT{d}") for d in range(8)]
        for d in range(8):
            dma_g(out=xT[d], in_=P["x0T"][d*128:(d+1)*128, :])

        # prefetch layer-l attention weights; issued one stage early so the
        # DMA runs during the previous layer's FFN.
        wtiles = {}
        def load_attn_weights(l):
            wkt = wp.tile([128, 8, 256], F16, tag="wkt", name=f"wkt{l}", bufs=1)
            wvt = wp.tile([128, 8, 256], F16, tag="wvt", name=f"wvt{l}", bufs=1)
            wqt = wp.tile([128, 8, 1024], F16, tag="wqt", name=f"wqt{l}", bufs=1)
            wot = wp.tile([128, 8, 1024], F16, tag="wot", name=f"wot{l}", bufs=1)
            wks = wp.tile([128, 8, 256], F16, tag="wks", name=f"wks{l}", bufs=1)
            wqs = wp.tile([128, 8, 1024], F16, tag="wqs", name=f"wqs{l}", bufs=1)
            dma(out=wkt, in_=P["wk"][l])
            dma(out=wvt, in_=P["wv"][l])
            dma(out=wqt, in_=P["wq"][l])
            dma_a(out=wot, in_=P["wo"][l])
            dma(out=wks, in_=P["wksw"][l])
            dma_a(out=wqs, in_=P["wqsw"][l])
            wtiles[l] = (wkt, wvt, wqt, wot, wks, wqs)
        cs = pers.tile([128, 256], F32, tag="cs", name="cs"); dma_g(out=cs, in_=P["cs"][:, :])
        sc = pers.tile([128, 256], F32, tag="sc", name="sc"); dma_g(out=sc, in_=P["sc"][:, :])
        mAg = pers.tile([128, 512], F16, tag="mAg", name="mAg"); dma_g(out=mAg, in_=P["mAg"][:, :])
        mBg = pers.tile([128, 512], F16, tag="mBg", name="mBg"); dma_g(out=mBg, in_=P["mBg"][:, :])
        mAl = pers.tile([128, 512], F16, tag="mAl", name="mAl"); dma_g(out=mAl, in_=P["mAl"][:, :])
        mBl = pers.tile([128, 768], F16, tag="mBl", name="mBl"); dma_g(out=mBl, in_=P["mBl"][:, :])
        # persistent V tiles (parity double-buffer); ones half memset once
        v_full2 = [pers.tile([128, 8, 4, 128], F16, tag=f"vfull{p_}", name=f"vfull{p_}")
                   for p_ in range(2)]
        for p_ in range(2):
            nc.gpsimd.memset(v_full2[p_][:, :, :, 64:128], 1.0)
        onesf = pers.tile([128, 1], F32, tag="onesf", name="onesf")
        nc.vector.memset(onesf, 1.0)
        ones = pers.tile([128, 1], F32R, tag="ones", name="ones")
        nc.vector.tensor_copy(out=ones, in_=onesf)
        ones1f = pers.tile([1, 128], F32, tag="ones1f", name="ones1f")
        nc.vector.memset(ones1f, 1.0)
        ones1r = pers.tile([1, 128], F32R, tag="ones1r", name="ones1r")
        nc.vector.tensor_copy(out=ones1r, in_=ones1f)
        epst = pers.tile([1, 1], F32, tag="epst", name="epst")
        nc.vector.memset(epst, EPS)

        def rmsnorm(nw_dram):
            nw = sm.tile([128, 8], F32, tag="nw", name="nw")
            dma_g(out=nw, in_=nw_dram)
            ss = ppm.tile([1, T], F32, tag="pm", name="ss")
            for d in range(8):
                x2 = act.tile([128, T], F32R, tag="x2", name="x2")
                nc.vector.tensor_mul(out=x2, in0=xT[d], in1=xT[d])
                nc.tensor.matmul(ss, lhsT=ones, rhs=x2, start=(d == 0), stop=(d == 7))
            rrow = sm.tile([1, T], F32, tag="rrow", name="rrow", bufs=2)
            nc.scalar.activation(out=rrow, in_=ss, func=AF.Sqrt, scale=1.0/D, bias=epst[0:1, 0:1])
            rrec = sm.tile([1, T], F32R, tag="rrec", name="rrec", bufs=2)
            if NO_APPROX:
                with nc.allow_low_precision(reason="rsqrt broadcast row, f32r for PE outer-product"):
                    nc.vector.reciprocal(out=rrec, in_=rrow)
            else:
                rraw = sm.tile([1, T], F32, tag="rraw", name="rraw", bufs=2)
                nc.vector.reciprocal_approx_fast(out=rraw, in_=rrow)
                with nc.allow_low_precision(reason="rsqrt broadcast row, f32r for PE outer-product"):
                    nc.vector.tensor_copy(out=rrec, in_=rraw)
            rb = ppm.tile([128, T], F32, tag="pm", name="rb")
            nc.tensor.matmul(rb, lhsT=ones1r, rhs=rrec, start=True, stop=True)
            out = []
            for d in range(8):
                h = act.tile([128, T], F16, tag=f"hT{d}", name=f"hT{d}", bufs=1)
                nc.vector.scalar_tensor_tensor(out=h, in0=xT[d], scalar=nw[:, d:d+1],
                                               in1=rb, op0=ALU.mult, op1=ALU.mult)
                out.append(h)
            return out

        def rope_combine(ps, ps_sw, outt):
            """qt = ps*COS + ps_sw*SS where ps_sw = swap32(ps) was produced
            directly by a second matmul with block-swapped weight columns."""
            u = sm.tile([128, 256], F32, tag="ropeu", name="ropeu", bufs=2)
            t = sm.tile([128, 256], F32, tag="ropet", name="ropet", bufs=2)
            nc.vector.tensor_mul(out=u, in0=ps, in1=cs)
            nc.vector.tensor_mul(out=t, in0=ps_sw, in1=sc)
            nc.vector.tensor_add(out=outt, in0=u, in1=t)

        def pe_filler(n):
            """Dependency-free f16 matmuls: execute during collective waits,
            keeping the PE HAM clock warm; result is discarded."""
            fd = ppm.tile([128, 512], F32, tag="pm", name="fill")
            for k_ in range(n):
                nc.tensor.matmul(fd, lhsT=mAg[:, 0:128], rhs=mAg[:, 0:512],
                                 start=(k_ == 0), stop=(k_ == n - 1))

        FFN_CHUNKS = [(0, 512), (512, 512), (1024, 512), (1536, 512), (2048, 512), (2560, 256)]
        ffn_tiles = {}
        def ffn_issue_chunk(l, c):
            h0, hwid = FFN_CHUNKS[c]
            w1t = wp.tile([128, 8, 512], F16, tag="w1t", name=f"w1t{l}_{c}")
            w3t = wp.tile([128, 8, 512], F16, tag="w3t", name=f"w3t{l}_{c}")
            dma(out=w1t[:, :, 0:hwid], in_=P["w1"][l][:, :, h0:h0+hwid])
            dma_a(out=w3t[:, :, 0:hwid], in_=P["w3"][l][:, :, h0:h0+hwid])
            ffn_tiles[(l, c)] = (w1t, w3t)
        w2_tiles = {}
        def w2_issue(l, hc):
            w2c = wp.tile([128, 512], F16, tag="w2c", name=f"w2c{l}_{hc}", bufs=3)
            dma_g(out=w2c, in_=P["w2"][l][:, hc, 0:512])
            w2_tiles[(l, hc)] = w2c
        w2hi_tiles = {}
        def w2hi_issue(l, hc):
            w2h_ = wp.tile([128, 512], F16, tag="w2h", name=f"w2h{l}_{hc}", bufs=4)
            dma(out=w2h_, in_=P["w2"][l][:, hc, 512:1024])
            w2hi_tiles[(l, hc)] = w2h_

        load_attn_weights(0)
        for l in range(NLAYERS):
            is_global = ((l + 1) % GEVERY) == 0
            wkt, wvt, wqt, wot, wks, wqs = wtiles.pop(l)
            h1 = rmsnorm(P["n1h"][l])
            # ---- K^T (direct), V ----
            for i in range(2):
                psk = ppm.tile([128, 256], F32, tag="pm", name="psk")
                for d in range(8):
                    nc.tensor.matmul(psk, lhsT=wkt[:, d, i*128:(i+1)*128], rhs=h1[d],
                                     start=(d == 0), stop=(d == 7))
                psk2 = ppm.tile([128, 256], F32, tag="pm", name="psk2")
                for d in range(8):
                    nc.tensor.matmul(psk2, lhsT=wks[:, d, i*128:(i+1)*128],
                                     rhs=h1[d], start=(d == 0), stop=(d == 7))
                kt = atp.tile([128, 256], F16, tag=f"ktc{i}", name=f"ktc{i}")
                rope_combine(psk, psk2, kt)
                dma_g(out=kv_in[l][i], in_=kt)
            for t2_ in range(2):
                psv = ppm.tile([128, 256], F32, tag="pm", name="psv")
                for d in range(8):
                    nc.tensor.matmul(psv, lhsT=h1[d][:, t2_*128:(t2_+1)*128], rhs=wvt[:, d, :],
                                     start=(d == 0), stop=(d == 7))
                vt = atp.tile([128, 256], F16, tag=f"vtok{t2_}", name=f"vtok{t2_}")
                nc.vector.tensor_copy(out=vt, in_=psv)
                dma_g(out=kv_in[l][2 + t2_], in_=vt)
            if NO_COLL:
                for r_ in range(4):
                    dma_g(out=kv_out[l][r_*4:(r_+1)*4], in_=kv_in[l][:])
            else:
                nc.gpsimd.collective_compute(
                    "AllGather", mybir.AluOpType.bypass, replica_groups=RG_KV,
                    ins=[kv_in[l].ap()], outs=[kv_out[l].ap()])
            # ---- Q^T (direct) ----
            qTk = [act.tile([128, 1024], F16, tag=f"qTk{i}", name=f"qTk{i}", bufs=1)
                   for i in range(2)]
            for t in range(8):
                i, j = t // 4, t % 4
                psq = ppm.tile([128, 256], F32, tag="pm", name="psq")
                for d in range(8):
                    nc.tensor.matmul(psq, lhsT=wqt[:, d, t*128:(t+1)*128], rhs=h1[d],
                                     start=(d == 0), stop=(d == 7))
                psq2 = ppm.tile([128, 256], F32, tag="pm", name="psq2")
                for d in range(8):
                    nc.tensor.matmul(psq2, lhsT=wqs[:, d, t*128:(t+1)*128],
                                     rhs=h1[d], start=(d == 0), stop=(d == 7))
                rope_combine(psq, psq2, qTk[i][:, j::4])
            # prefetch FFN weights while attention runs
            if not SKIP_FFN:
                ffn_issue_chunk(l, 0)
                ffn_issue_chunk(l, 1)
                for hc_ in range(3):
                    w2_issue(l, hc_)
            # ---- gather K/V from collective ----
            kT_full = [atp.tile([128, 1024], F16, tag=f"kTf{i}", name=f"kTf{i}", bufs=1) for i in range(2)]
            for i in range(2):
                dma_g(out=kT_full[i].rearrange("p (r t) -> p r t", r=4),
                    in_=kv_out[l][i::4].rearrange("r p t -> p r t"))
            v_full = v_full2[l % 2]
            for sl in range(2):
                for r in range(4):
                    dma_g(out=v_full[:, 2*r+sl, :, 0:64],
                        in_=kv_out[l][r*4+2+sl].rearrange("p (f h) -> p f h", h=64))
            if not NO_FILL:
                pe_filler(24)
            # ---- attention (4 q heads batched per kv head; sl pairs
            # interleaved so row-groups 0-63/64-127 run concurrently) ----
            oTk = [act.tile([128, 1024], F16, tag=f"oTk{i}", name=f"oTk{i}", bufs=1)
                   for i in range(2)]
            for i in range(2 if not SKIP_ATTN else 0):
                for qb in range(2):
                    if qb == 0:
                        groups = [([0, 1, 2, 3], mAg if is_global else mAl)]
                    elif is_global:
                        groups = [([0, 1, 2, 3], None), ([4, 5, 6, 7], mBg)]
                    else:
                        groups = [([2, 3, 4, 5], mBl[:, 0:512]), ([6, 7], mBl[:, 512:768])]
                    nblk = sum(len(g[0]) for g in groups)
                    pts = {0: [], 1: []}
                    for (blocks, msk) in groups:
                        for gi, b in enumerate(blocks):
                            for sl in range(2):
                                base = sl * 64
                                psS = pps.tile([128, 512], F32, tag="pps", name="psS")
                                nc.tensor.matmul(psS,
                                                 lhsT=kT_full[i][base:base+64, JOF[b]*128:JOF[b]*128+128],
                                                 rhs=qTk[i][base:base+64, qb*512:(qb+1)*512],
                                                 start=True, stop=True)
                                if msk is not None:
                                    mb_ = msk[:, gi*128:(gi+1)*128].rearrange(
                                        "p (q o) -> p q o", o=1).to_broadcast((128, 128, 4))
                                    nc.vector.tensor_add(
                                        out=psS.rearrange("p (q o) -> p q o", o=4),
                                        in0=psS.rearrange("p (q o) -> p q o", o=4), in1=mb_)
                                pt = atp.tile([128, 512], F16, tag="pt", name="pt", bufs=4)
                                nc.scalar.activation(out=pt, in_=psS, func=AF.Exp, scale=SCALE)
                                pts[sl].append((b, pt))
                    psO = {}
                    for sl in range(2):
                        psO[sl] = ppo.tile([128, 512], F32, tag="ppo", name="psO")
                    for bi in range(nblk):
                        for sl in range(2):
                            b, pt = pts[sl][bi]
                            nc.tensor.matmul(psO[sl],
                                             lhsT=v_full[:, JOF[b], 2*i+sl, :],
                                             rhs=pt,
                                             start=(bi == 0), stop=(bi == nblk - 1))
                    for sl in range(2):
                        base = sl * 64
                        rec = sm.tile([64, 512], F32, tag="rec", name="rec", bufs=2)
                        if APPROX_SBUF:
                            den = sm.tile([64, 512], F32, tag="den", name="den", bufs=2)
                            nc.scalar.activation(out=den, in_=psO[sl][64:128, :], func=AF.Copy)
                            nc.vector.reciprocal_approx_fast(out=rec, in_=den)
                        elif NO_APPROX:
                            nc.vector.reciprocal(out=rec, in_=psO[sl][64:128, :])
                        else:
                            nc.vector.reciprocal_approx_fast(out=rec, in_=psO[sl][64:128, :])
                        nc.vector.tensor_mul(out=oTk[i][base:base+64, qb*512:(qb+1)*512],
                                             in0=psO[sl][0:64, :], in1=rec)
            # ---- O proj ----
            for d in range(8):
                pso = ppm.tile([128, 256], F32, tag="pm", name="pso")
                for ft in range(8):
                    nc.tensor.matmul(pso, lhsT=wot[:, ft, d*128:(d+1)*128],
                                     rhs=oTk[ft // 4][:, ft % 4::4],
                                     start=(ft == 0), stop=(ft == 7))
                nc.vector.tensor_add(out=xT[d], in0=xT[d], in1=pso)
            # prefetch next layer's attention weights during this FFN
            if l + 1 < NLAYERS:
                load_attn_weights(l + 1)
            # ---- FFN (producer/consumer interleaved per 128-col chunk) ----
            if SKIP_FFN:
                continue
            h2 = rmsnorm(P["n2h"][l])
            # phase-1 accumulators for d 0-3 (one chain per PSUM bank; a bank
            # cannot host two start/stop chains - start zeroes the whole row)
            banks = [pps.tile([128, 256], F32, tag="pps", name=f"fb{k}") for k in range(2)] \
                  + [ppo.tile([128, 256], F32, tag="ppo", name=f"fb{k+2}") for k in range(2)]
            yT = {}
            pend = []          # hc awaiting phase-1 w2 consumption
            def w2_consume():
                hc = pend.pop(0)
                w2c = w2_tiles.pop((l, hc))
                for d in range(4):
                    nc.tensor.matmul(banks[d],
                                     lhsT=w2c[:, d*128:(d+1)*128], rhs=yT[hc],
                                     start=(hc == 0), stop=(hc == 21))
            for c, (h0, hwid) in enumerate(FFN_CHUNKS):
                if c + 2 < len(FFN_CHUNKS):
                    ffn_issue_chunk(l, c + 2)
                w1t, w3t = ffn_tiles.pop((l, c))
                for hj in range(0, hwid, 128):
                    hc = (h0 + hj) // 128
                    if hc + 3 < 22:
                        w2_issue(l, hc + 3)
                    psu = ppm.tile([128, 256], F32, tag="pm", name="psu")
                    psg = ppm.tile([128, 256], F32, tag="pm", name="psg")
                    for d in range(8):
                        nc.tensor.matmul(psu, lhsT=w1t[:, d, hj:hj+128], rhs=h2[d],
                                         start=(d == 0), stop=(d == 7))
                    for d in range(8):
                        nc.tensor.matmul(psg, lhsT=w3t[:, d, hj:hj+128], rhs=h2[d],
                                         start=(d == 0), stop=(d == 7))
                    su = act.tile([128, 256], F32, tag="su", name="su")
                    y = act.tile([128, 256], F16, tag=f"yT{hc}", name=f"yT{hc}", bufs=1)
                    if SIM_SILU:
                        nc.scalar.activation(out=su, in_=psu, func=AF.Sigmoid)
                        nc.vector.tensor_mul(out=su, in0=su, in1=psu)
                        nc.vector.tensor_mul(out=y, in0=su, in1=psg)
                    else:
                        nc.scalar.activation(out=su, in_=psu, func=AF.Silu)
                        nc.vector.tensor_mul(out=y, in0=su, in1=psg)
                    yT[hc] = y
                    pend.append(hc)
                    if len(pend) > 1:
                        w2_consume()
            while pend:
                w2_consume()
            for d in range(4):
                nc.vector.tensor_add(out=xT[d], in0=xT[d], in1=banks[d])
            # phase 2: d 4-7 over all 22 chunks (hi half of w2 streamed on sync q)
            banks2 = [pps.tile([128, 256], F32, tag="pps", name=f"fb2{k}") for k in range(2)] \
                   + [ppo.tile([128, 256], F32, tag="ppo", name=f"fb2{k+2}") for k in range(2)]
            for hc_ in range(4):
                w2hi_issue(l, hc_)
            for hc in range(22):
                if hc + 4 < 22:
                    w2hi_issue(l, hc + 4)
                w2h_ = w2hi_tiles.pop((l, hc))
                for dh in range(4):
                    nc.tensor.matmul(banks2[dh],
                                     lhsT=w2h_[:, dh*128:(dh+1)*128], rhs=yT[hc],
                                     start=(hc == 0), stop=(hc == 21))
            for dh in range(4):
                nc.vector.tensor_add(out=xT[4+dh], in0=xT[4+dh], in1=banks2[dh])
        # ---- final ----
        xf = rmsnorm(P["nfh"][:, :])
        for h_ in range(2):
            for dd in range(4):
                d = h_ * 4 + dd
                dma_g(out=xf_in[h_][:, dd*256:(dd+1)*256], in_=xf[d])
            if NO_COLL:
                for r_ in range(8):
                    dma_g(out=xf_out[h_][r_*128:(r_+1)*128, :], in_=xf_in[h_][:, :])
            else:
                nc.gpsimd.collective_compute(
                    "AllGather", mybir.AluOpType.bypass, replica_groups=RG_ALL,
                    ins=[xf_in[h_].ap()], outs=[xf_out[h_].ap()])
        if not NO_FILL:
            pe_filler(120)
        xfT = []
        XF_TAGS = [("wkt", 1), ("wvt", 1), ("w1t", 2), ("w1t", 2),
                   ("w3t", 2), ("w3t", 2), ("w2c", 3), ("w2c", 3)]
        for r in range(8):
            xt = wp.tile([128, 2, 4, 256], F16, tag=XF_TAGS[r][0], bufs=XF_TAGS[r][1], name=f"xfT{r}")
            for h_ in range(2):
                dma_g(out=xt[:, h_], in_=xf_out[h_][r*128:(r+1)*128, :].rearrange("p (d t) -> p d t", t=256))
            xfT.append(xt)
        for vc in range(8 if not SKIP_LM else 0):
            embt = wp.tile([128, 8, 500], F16, tag=("wqt", "wot")[vc % 2], bufs=1, name="embt")
            dma_a(out=embt, in_=P["embT"][:, :, vc*500:(vc+1)*500])
            for r in range(8):
                lg = act.tile([128, 2, 500], F16, tag="lg", name="lg", bufs=2)
                for tch in range(2):
                    psl = pps.tile([128, 500], F32, tag="pps", name="psl")
                    for d in range(8):
                        nc.tensor.matmul(psl, lhsT=xfT[r][:, d // 4, d % 4, tch*128:(tch+1)*128],
                                         rhs=embt[:, d, :], start=(d == 0), stop=(d == 7))
                    if (r + tch) % 2 == 0:
                        nc.vector.tensor_copy(out=lg[:, tch, :], in_=psl)
                    else:
                        nc.scalar.activation(out=lg[:, tch, :], in_=psl, func=AF.Copy)
                if not (HALF_LGWR and vc % 2 == 1):
                    dma(out=logits[r*256:(r+1)*256, vc*500:(vc+1)*500].rearrange("(c p) v -> p c v", p=128),
                        in_=lg)
    nc.compile()
    return nc

_NC_CACHE = {}
def _get_nc():
    if "nc" not in _NC_CACHE:
        _NC_CACHE["nc"] = _build_nc()
    return _NC_CACHE["nc"]

def kernel(**inputs):
    from concourse.bass_utils import run_bass_kernel_spmd
    nc = _get_nc()
    in_maps = _host_prep(**inputs)
    res = run_bass_kernel_spmd(nc, in_maps, list(range(NCORES)))
    return _assemble(res.results)

